# revision 5
# baseline (speedup 1.0000x reference)
"""Bass/Trainium2 kernel for nn_CCELossFast (calibration-histogram SCE loss).

Math: reference computes softmax probs p[r,c] over C=1000 classes for
B=262144 rows, bins each p into 10 confidence bins, builds per-(class,bin)
tables no_pred / no_acc / conf_sum, and returns
    loss = sum_{c,b} |no_acc - conf| * n/(n+eps) / sum(no_pred).
In f32, n/(n+eps) == 1 exactly for n >= 1 and empty cells contribute 0, so
    loss = sum_{c,b} |no_acc[c,b] - conf_sum[c,b]| / (B*C).
With standard-normal logits, p > 0.1 (i.e. any bin other than 0) happens for
only ~tens of elements in the whole dataset, and at most one element per row
(it must be the row max).  So:
  device (per core, data-parallel over rows): e = exp(x); s[r] = sum_c e;
    colsum[c] = sum_r e[r,c]/s[r]  (accumulating matmul with lhsT = 1/s);
    also emit per-row s and max_c e so the host can find the rare rows with
    p_max = max_e/s > 0.1.
  host: D[c,0] = bincount(target)[c] - colsum[c]; for each flagged row,
    recompute that row's f32 softmax exactly like the reference and move its
    >bin-0 elements' (onehot - p) contributions from bin 0 to their true bin;
    loss = sum |D| / (B*C).
"""

import numpy as np

N_CORES = 8
B_TOTAL = 262144
C = 1000
P = 128

FULL_ROWS = B_TOTAL // N_CORES  # 32768 rows per core
DEFAULT_GROUP = 8

# float32 bin bounds, identical to jnp.linspace(0.0, 1.0, 11).astype(f32)
# (differs from np.linspace(...).astype(f32) in the last ulp for some entries)
BOUNDS = np.array(
    [
        0.0,
        0.10000000149011612,
        0.20000000298023224,
        0.30000001192092896,
        0.4000000059604645,
        0.5,
        0.6000000238418579,
        0.699999988079071,
        0.800000011920929,
        0.9000000357627869,
        1.0,
    ],
    dtype=np.float32,
)


def emit_body(tc, x_ap, colsum_ap, s_ap, m_ap, rows, group=DEFAULT_GROUP):
    """Emit the per-core Tile program. x:[rows,C] f32 in DRAM; outputs
    colsum:[1,C] f32, s:[P,ntiles] f32, m:[P,ntiles] f32."""
    import concourse.mybir as mybir

    nc = tc.nc
    FP32 = mybir.dt.float32
    BF16 = mybir.dt.bfloat16
    ntiles = rows // P
    assert rows % P == 0 and ntiles % group == 0
    H0 = 512  # psum bank split: [0:512], [512:1000]

    with (
        tc.tile_pool(name="xp", bufs=4) as xp,
        tc.tile_pool(name="ep", bufs=2 * group + 2) as ep,
        tc.tile_pool(name="stat", bufs=1) as statp,
        tc.tile_pool(name="psump", bufs=1, space="PSUM") as psp,
    ):
        s_stage = statp.tile([P, ntiles], FP32, tag="s")
        m_stage = statp.tile([P, ntiles], FP32, tag="m")
        r_stage = statp.tile([P, ntiles], FP32, tag="r")
        rb_stage = statp.tile([P, ntiles], BF16, tag="rb")
        out_sb = statp.tile([1, C], FP32, tag="o")
        ps = psp.tile([1, C], FP32, tag="ps")

        for g in range(ntiles // group):
            es = []
            for j in range(group):
                t = g * group + j
                xt = xp.tile([P, C], FP32, tag="x")
                nc.gpsimd.dma_start(xt[:], x_ap[t * P : (t + 1) * P, :])
                et = ep.tile([P, C], BF16, tag="e")
                nc.scalar.activation(
                    et[:],
                    xt[:],
                    mybir.ActivationFunctionType.Exp,
                    accum_out=s_stage[:, t : t + 1],
                )
                nc.vector.reduce_max(
                    m_stage[:, t : t + 1], et[:], axis=mybir.AxisListType.X
                )
                es.append(et)
            gs = slice(g * group, (g + 1) * group)
            nc.vector.reciprocal(r_stage[:, gs], s_stage[:, gs])
            nc.vector.tensor_copy(rb_stage[:, gs], r_stage[:, gs])
            for j in range(group):
                t = g * group + j
                for lo, hi in ((0, H0), (H0, C)):
                    nc.tensor.matmul(
                        ps[0:1, lo:hi],
                        lhsT=rb_stage[:, t : t + 1],
                        rhs=es[j][:, lo:hi],
                        start=(t == 0),
                        stop=(t == ntiles - 1),
                    )
        nc.vector.tensor_copy(out_sb[:], ps[:])
        nc.sync.dma_start(colsum_ap[:, :], out_sb[:])
        nc.sync.dma_start(s_ap[:, :], s_stage[:])
        nc.sync.dma_start(m_ap[:, :], m_stage[:])


def build_nc(rows=FULL_ROWS, group=DEFAULT_GROUP):
    import concourse.bacc as bacc
    import concourse.mybir as mybir
    from concourse import tile

    FP32 = mybir.dt.float32
    ntiles = rows // P
    nc = bacc.Bacc(
        "TRN2", target_bir_lowering=False, debug=False, num_devices=N_CORES
    )
    x = nc.dram_tensor("x", [rows, C], FP32, kind="ExternalInput").ap()
    colsum = nc.dram_tensor("colsum", [1, C], FP32, kind="ExternalOutput").ap()
    s_out = nc.dram_tensor("s_out", [P, ntiles], FP32, kind="ExternalOutput").ap()
    m_out = nc.dram_tensor("m_out", [P, ntiles], FP32, kind="ExternalOutput").ap()
    with tile.TileContext(nc) as tc:
        emit_body(tc, x, colsum, s_out, m_out, rows, group)
    nc.compile()
    return nc


def run_device(output, trace=False):
    """Shard rows across 8 cores, run the bass kernel, return per-core results
    and (if trace) hardware exec time in ns."""
    from concourse.bass_utils import run_bass_kernel_spmd

    nc = build_nc()
    in_maps = [
        {"x": output[c * FULL_ROWS : (c + 1) * FULL_ROWS]} for c in range(N_CORES)
    ]
    res = run_bass_kernel_spmd(nc, in_maps, list(range(N_CORES)), trace=trace)
    return res


def _host_reduce(output, target, results):
    target = np.asarray(target).astype(np.int64)
    colsum = np.zeros(C, dtype=np.float64)
    for c in range(N_CORES):
        colsum += results[c]["colsum"][0].astype(np.float64)
    count = np.bincount(target, minlength=C).astype(np.float64)

    D = np.zeros((C, 10), dtype=np.float64)
    D[:, 0] = count - colsum

    cls_idx = np.arange(C)
    for c in range(N_CORES):
        s = results[c]["s_out"]  # [P, ntiles]
        me = results[c]["m_out"]
        pmax = me / s
        for pp, tt in np.argwhere(pmax > 0.09):
            r_local = tt * P + pp
            r_global = c * FULL_ROWS + r_local
            xr = np.asarray(output[r_global], dtype=np.float32)
            e = np.exp(xr - xr.max(), dtype=np.float32)
            p = (e / e.sum(dtype=np.float32)).astype(np.float32)
            binv = np.clip(np.searchsorted(BOUNDS, p, side="left") - 1, 0, 9)
            movers = np.where(binv >= 1)[0]
            for ci in movers:
                v = float(target[r_global] == cls_idx[ci]) - np.float64(p[ci])
                D[ci, 0] -= v
                D[ci, binv[ci]] += v

    loss = np.abs(D).sum() / float(B_TOTAL) / float(C)
    return np.float32(loss)


def kernel(output, target):
    output = np.asarray(output)
    res = run_device(output, trace=False)
    return _host_reduce(output, target, res.results)


# revision 7
# speedup vs baseline: 1.3483x; 1.3483x over previous
"""Bass/Trainium2 kernel for nn_CCELossFast (calibration-histogram SCE loss).

Math: reference computes softmax probs p[r,c] over C=1000 classes for
B=262144 rows, bins each p into 10 confidence bins, builds per-(class,bin)
tables no_pred / no_acc / conf_sum, and returns
    loss = sum_{c,b} |no_acc - conf| * n/(n+eps) / sum(no_pred).
In f32, n/(n+eps) == 1 exactly for n >= 1 and empty cells contribute 0, so
    loss = sum_{c,b} |no_acc[c,b] - conf_sum[c,b]| / (B*C).
With standard-normal logits, p > 0.1 (i.e. any bin other than 0) happens for
only ~tens of elements in the whole dataset, and at most one element per row
(it must be the row max).  So:
  device (per core, data-parallel over rows): e = exp(x); s[r] = sum_c e;
    colsum[c] = sum_r e[r,c]/s[r]  (accumulating matmul with lhsT = 1/s);
    also emit per-row s and max_c e so the host can find the rare rows with
    p_max = max_e/s > 0.1.
  host: D[c,0] = bincount(target)[c] - colsum[c]; for each flagged row,
    recompute that row's f32 softmax exactly like the reference and move its
    >bin-0 elements' (onehot - p) contributions from bin 0 to their true bin;
    loss = sum |D| / (B*C).
"""

import numpy as np

N_CORES = 8
B_TOTAL = 262144
C = 1000
P = 128

FULL_ROWS = B_TOTAL // N_CORES  # 32768 rows per core
DEFAULT_GROUP = 8

# float32 bin bounds, identical to jnp.linspace(0.0, 1.0, 11).astype(f32)
# (differs from np.linspace(...).astype(f32) in the last ulp for some entries)
BOUNDS = np.array(
    [
        0.0,
        0.10000000149011612,
        0.20000000298023224,
        0.30000001192092896,
        0.4000000059604645,
        0.5,
        0.6000000238418579,
        0.699999988079071,
        0.800000011920929,
        0.9000000357627869,
        1.0,
    ],
    dtype=np.float32,
)


def emit_body(tc, x_ap, colsum_ap, s_ap, m_ap, rows, group=DEFAULT_GROUP):
    """Emit the per-core Tile program. x:[rows,C] f32 in DRAM; outputs
    colsum:[1,C] f32, s:[P,ntiles] f32, m:[P,ntiles] f32."""
    import concourse.mybir as mybir

    nc = tc.nc
    FP32 = mybir.dt.float32
    BF16 = mybir.dt.bfloat16
    ntiles = rows // P
    assert rows % P == 0 and ntiles % group == 0
    H0 = 512  # psum bank split: [0:512], [512:1000]

    with (
        tc.tile_pool(name="xp", bufs=6) as xp,
        tc.tile_pool(name="ep", bufs=2 * group + 2) as ep,
        tc.tile_pool(name="stat", bufs=1) as statp,
        tc.tile_pool(name="psump", bufs=1, space="PSUM") as psp,
    ):
        s_stage = statp.tile([P, ntiles], FP32, tag="s")
        m_stage = statp.tile([P, ntiles], FP32, tag="m")
        r_stage = statp.tile([P, ntiles], FP32, tag="r")
        rb_stage = statp.tile([P, ntiles], BF16, tag="rb")
        out_sb = statp.tile([1, C], FP32, tag="o")
        ps = psp.tile([1, C], FP32, tag="ps")

        for g in range(ntiles // group):
            es = []
            for j in range(group):
                t = g * group + j
                xt = xp.tile([P, C], FP32, tag="x")
                nc.sync.dma_start(xt[:], x_ap[t * P : (t + 1) * P, :])
                et = ep.tile([P, C], BF16, tag="e")
                nc.scalar.activation(
                    et[:],
                    xt[:],
                    mybir.ActivationFunctionType.Exp,
                    accum_out=s_stage[:, t : t + 1],
                )
                nc.vector.reduce_max(
                    m_stage[:, t : t + 1], et[:], axis=mybir.AxisListType.X
                )
                es.append(et)
            gs = slice(g * group, (g + 1) * group)
            nc.vector.reciprocal(r_stage[:, gs], s_stage[:, gs])
            nc.vector.tensor_copy(rb_stage[:, gs], r_stage[:, gs])
            for j in range(group):
                t = g * group + j
                for lo, hi in ((0, H0), (H0, C)):
                    nc.tensor.matmul(
                        ps[0:1, lo:hi],
                        lhsT=rb_stage[:, t : t + 1],
                        rhs=es[j][:, lo:hi],
                        start=(t == 0),
                        stop=(t == ntiles - 1),
                    )
        nc.vector.tensor_copy(out_sb[:], ps[:])
        nc.sync.dma_start(colsum_ap[:, :], out_sb[:])
        nc.sync.dma_start(s_ap[:, :], s_stage[:])
        nc.sync.dma_start(m_ap[:, :], m_stage[:])


def build_nc(rows=FULL_ROWS, group=DEFAULT_GROUP):
    import concourse.bacc as bacc
    import concourse.mybir as mybir
    from concourse import tile

    FP32 = mybir.dt.float32
    ntiles = rows // P
    nc = bacc.Bacc(
        "TRN2", target_bir_lowering=False, debug=False, num_devices=N_CORES
    )
    x = nc.dram_tensor("x", [rows, C], FP32, kind="ExternalInput").ap()
    colsum = nc.dram_tensor("colsum", [1, C], FP32, kind="ExternalOutput").ap()
    s_out = nc.dram_tensor("s_out", [P, ntiles], FP32, kind="ExternalOutput").ap()
    m_out = nc.dram_tensor("m_out", [P, ntiles], FP32, kind="ExternalOutput").ap()
    with tile.TileContext(nc) as tc:
        emit_body(tc, x, colsum, s_out, m_out, rows, group)
    nc.compile()
    return nc


def run_device(output, trace=False):
    """Shard rows across 8 cores, run the bass kernel, return per-core results
    and (if trace) hardware exec time in ns."""
    from concourse.bass_utils import run_bass_kernel_spmd

    nc = build_nc()
    in_maps = [
        {"x": output[c * FULL_ROWS : (c + 1) * FULL_ROWS]} for c in range(N_CORES)
    ]
    res = run_bass_kernel_spmd(nc, in_maps, list(range(N_CORES)), trace=trace)
    return res


def _host_reduce(output, target, results):
    target = np.asarray(target).astype(np.int64)
    colsum = np.zeros(C, dtype=np.float64)
    for c in range(N_CORES):
        colsum += results[c]["colsum"][0].astype(np.float64)
    count = np.bincount(target, minlength=C).astype(np.float64)

    D = np.zeros((C, 10), dtype=np.float64)
    D[:, 0] = count - colsum

    cls_idx = np.arange(C)
    for c in range(N_CORES):
        s = results[c]["s_out"]  # [P, ntiles]
        me = results[c]["m_out"]
        pmax = me / s
        for pp, tt in np.argwhere(pmax > 0.09):
            r_local = tt * P + pp
            r_global = c * FULL_ROWS + r_local
            xr = np.asarray(output[r_global], dtype=np.float32)
            e = np.exp(xr - xr.max(), dtype=np.float32)
            p = (e / e.sum(dtype=np.float32)).astype(np.float32)
            binv = np.clip(np.searchsorted(BOUNDS, p, side="left") - 1, 0, 9)
            movers = np.where(binv >= 1)[0]
            for ci in movers:
                v = float(target[r_global] == cls_idx[ci]) - np.float64(p[ci])
                D[ci, 0] -= v
                D[ci, binv[ci]] += v

    loss = np.abs(D).sum() / float(B_TOTAL) / float(C)
    return np.float32(loss)


def kernel(output, target):
    output = np.asarray(output)
    res = run_device(output, trace=False)
    return _host_reduce(output, target, res.results)
